# revision 6
# baseline (speedup 1.0000x reference)
"""Trainium2 Bass kernel for CircularNN (3 sparse gather-layers + dense head + softmax).

Strategy:
  - Pure data parallel over batch: 65536 rows -> 8 cores x 8192.
  - The sparse layers out[b,n] = sum_j x[b, idx[n,j]]*w[n,j] + b[n] have
    *fixed* index tables, so each layer is x @ W_dense for a scatter-built
    dense [784,784] matrix (host builds the matrix; all FLOPs on device).
  - Activations are kept feature-major (transposed) on chip so the tensor
    engine contracts along partitions; x is shipped pre-transposed.
  - Matmuls run in float32r (full fp32-width operands, 1 col/cycle when
    N>=256 vs 4 cycles/row for plain fp32).
  - GELU (exact, erf-based) fused with the per-feature bias on ScalarE,
    reading PSUM directly.
  - Softmax without any ACT table switch: exp(l) = (1+t)/(1-t) with
    t = tanh(l/2) (tanh lives in the same LUT set as gelu); the 10-way
    partition reduction and the broadcast run as tiny PE matmuls.
"""

import os
import sys
import types

sys.path.insert(0, "/opt/trn_rl_repo")

import numpy as np


def _ensure_axon_hooks():
    """concourse's trace path imports antenv.axon_hooks, which this image
    lacks. Provide it (and register the NTFF profile hook when possible) so
    trace=True/BASS_TRACE=1 works instead of crashing."""
    try:
        import antenv
    except ImportError:
        return
    if "antenv.axon_hooks" in sys.modules:
        return
    hooks = types.ModuleType("antenv.axon_hooks")
    hooks._hook = None

    def set_axon_ntff_profile_hook(h):
        hooks._hook = h

    def get_axon_ntff_profile_hook():
        return hooks._hook

    hooks.set_axon_ntff_profile_hook = set_axon_ntff_profile_hook
    hooks.get_axon_ntff_profile_hook = get_axon_ntff_profile_hook
    sys.modules["antenv.axon_hooks"] = hooks
    antenv.axon_hooks = hooks
    try:
        from trn_agent_boot.trn_boot import _ntff_profile_via_ctypes

        hook = _ntff_profile_via_ctypes("/opt/axon/libaxon_pjrt.so")
        if hook is not None:
            hooks._hook = hook
    except Exception:
        pass


_ensure_axon_hooks()

B, D, NCLS = 65536, 784, 10
NCORES = 8
BS = B // NCORES  # 8192 rows per core
CHUNK = 512  # batch columns per tile pass (fp32 moving-operand max)
NCH = BS // CHUNK  # 16
P = 112  # partition tile: 784 = 7 * 112
KT = D // P  # 7 contraction/output tiles

_CACHE = {}


def _build_program():
    from concourse import bacc, bass, mybir, tile

    f32 = mybir.dt.float32
    f32r = mybir.dt.float32r
    AF = mybir.ActivationFunctionType
    ALU = mybir.AluOpType

    nc = bacc.Bacc("TRN2", target_bir_lowering=False, debug=False)

    xt = nc.dram_tensor("xt", [D, BS], f32, kind="ExternalInput")
    wd = [
        nc.dram_tensor(f"w{i}", [D, D], f32, kind="ExternalInput") for i in (1, 2, 3)
    ]
    bd = [nc.dram_tensor(f"b{i}", [D], f32, kind="ExternalInput") for i in (1, 2, 3)]
    fcw = nc.dram_tensor("fcwt", [D, NCLS], f32, kind="ExternalInput")
    fcbh = nc.dram_tensor("fcbh", [NCLS, 1], f32, kind="ExternalInput")
    onesd = nc.dram_tensor("onesd", [NCLS, 1], f32, kind="ExternalInput")
    out_t = nc.dram_tensor("out_t", [NCLS, BS], f32, kind="ExternalOutput")

    with tile.TileContext(nc) as tc:
        with (
            tc.tile_pool(name="const", bufs=1) as cpool,
            tc.tile_pool(name="xc", bufs=3) as xpool,
            tc.tile_pool(name="hbuf", bufs=1) as hpool,
            tc.tile_pool(name="eps", bufs=2) as epool,
            tc.tile_pool(name="ps", bufs=3, space="PSUM") as pspool,
            tc.tile_pool(name="psfc", bufs=2, space="PSUM") as fcpool,
            tc.tile_pool(name="pssm", bufs=1, space="PSUM") as smpool,
        ):
            # ---- persistent weights / tables ----
            ws = []
            for i in range(3):
                t = cpool.tile([P, KT, D], f32r, tag=f"w{i}")
                nc.sync.dma_start(
                    out=t[:], in_=wd[i][:].rearrange("(kt p) n -> p kt n", p=P).bitcast(f32r)
                )
                ws.append(t)
            bs = []
            for i in range(3):
                t = cpool.tile([P, KT], f32, tag=f"b{i}")
                nc.sync.dma_start(
                    out=t[:], in_=bd[i][:].rearrange("(m p) -> p m", p=P)
                )
                bs.append(t)
            fcws = cpool.tile([P, KT, NCLS], f32r, tag="fcw")
            nc.sync.dma_start(
                out=fcws[:], in_=fcw[:].rearrange("(kt p) n -> p kt n", p=P).bitcast(f32r)
            )
            fcbhs = cpool.tile([NCLS, 1], f32, tag="fcbh")
            nc.sync.dma_start(out=fcbhs[:], in_=fcbh[:])
            ones_k = cpool.tile([NCLS, 1], f32r, tag="ones_k")
            nc.sync.dma_start(out=ones_k[:], in_=onesd[:].bitcast(f32r))
            ones_m = cpool.tile([1, NCLS], f32r, tag="ones_m")
            nc.sync.dma_start(
                out=ones_m[:], in_=onesd[:].rearrange("n o -> o n").bitcast(f32r)
            )

            xt_r = xt[:].rearrange("(kt p) b -> p kt b", p=P)

            for c in range(NCH):
                c0 = c * CHUNK
                xc = xpool.tile([P, KT, CHUNK], f32r, tag="xc")
                nc.sync.dma_start(out=xc[:], in_=xt_r[:, :, c0 : c0 + CHUNK].bitcast(f32r))

                src = xc
                for L in range(3):
                    h = hpool.tile([P, KT, CHUNK], f32r, tag=f"h{L}")
                    for m in range(KT):
                        ps = pspool.tile([P, CHUNK], f32, tag="ps")
                        for kt in range(KT):
                            nc.tensor.matmul(
                                ps[:],
                                ws[L][:, kt, m * P : (m + 1) * P],
                                src[:, kt, :],
                                start=(kt == 0),
                                stop=(kt == KT - 1),
                            )
                        nc.scalar.activation(
                            h[:, m, :], ps[:], AF.Gelu, bias=bs[L][:, m : m + 1]
                        )
                    src = h

                # dense head: logits^T = fc_w @ h3^T   -> [10, CHUNK] psum
                psfc = fcpool.tile([NCLS, CHUNK], f32, tag="psfc")
                for kt in range(KT):
                    nc.tensor.matmul(
                        psfc[:],
                        fcws[:, kt, :],
                        src[:, kt, :],
                        start=(kt == 0),
                        stop=(kt == KT - 1),
                    )
                # softmax: e = exp(l) = (1+t)/(1-t), t = tanh(l/2)
                t_sb = epool.tile([NCLS, CHUNK], f32, tag="t")
                nc.scalar.activation(
                    t_sb[:], psfc[:], AF.Tanh, bias=fcbhs[:], scale=0.5
                )
                u = epool.tile([NCLS, CHUNK], f32, tag="u")
                nc.vector.tensor_scalar(u[:], t_sb[:], -1.0, 1.0, ALU.mult, ALU.add)
                r = epool.tile([NCLS, CHUNK], f32, tag="r")
                nc.vector.reciprocal(r[:], u[:])
                v = epool.tile([NCLS, CHUNK], f32, tag="v")
                nc.vector.tensor_scalar_add(v[:], t_sb[:], 1.0)
                e = epool.tile([NCLS, CHUNK], f32r, tag="e")
                with nc.allow_low_precision(reason="f32r is fp32-width for PE"):
                    nc.vector.tensor_mul(e[:], v[:], r[:])
                # denom = sum over the 10 classes (partition axis) via PE
                pss = smpool.tile([1, CHUNK], f32, tag="pss")
                nc.tensor.matmul(pss[:], ones_k[:], e[:])
                rs = epool.tile([1, CHUNK], f32r, tag="rs")
                with nc.allow_low_precision(reason="f32r is fp32-width for PE"):
                    nc.vector.reciprocal(rs[:], pss[:])
                psb = smpool.tile([NCLS, CHUNK], f32, tag="psb")
                nc.tensor.matmul(psb[:], ones_m[:], rs[:])
                prob = epool.tile([NCLS, CHUNK], f32, tag="prob")
                nc.vector.tensor_mul(prob[:], e[:].bitcast(f32), psb[:])
                nc.sync.dma_start(out=out_t[:, c0 : c0 + CHUNK], in_=prob[:])

    nc.compile()
    return nc


def _get_program():
    if "nc" not in _CACHE:
        _CACHE["nc"] = _build_program()
    return _CACHE["nc"]


def _dense_weight(idx: np.ndarray, w: np.ndarray) -> np.ndarray:
    """Scatter the [Dout, k] index/weight tables into a dense [Din, Dout]."""
    wd = np.zeros((D, D), dtype=np.float32)
    cols = np.arange(D)
    for j in range(idx.shape[1]):
        np.add.at(wd, (idx[:, j], cols), w[:, j])
    return wd


def kernel(**inputs) -> np.ndarray:
    x = np.asarray(inputs["x"], dtype=np.float32)
    idx1 = np.asarray(inputs["idx1"])
    idx2 = np.asarray(inputs["idx2"])
    idx3 = np.asarray(inputs["idx3"])
    w1 = np.asarray(inputs["w1"], dtype=np.float32)
    w2 = np.asarray(inputs["w2"], dtype=np.float32)
    w3 = np.asarray(inputs["w3"], dtype=np.float32)
    b1 = np.asarray(inputs["b1"], dtype=np.float32)
    b2 = np.asarray(inputs["b2"], dtype=np.float32)
    b3 = np.asarray(inputs["b3"], dtype=np.float32)
    fc_w = np.asarray(inputs["fc_w"], dtype=np.float32)
    fc_b = np.asarray(inputs["fc_b"], dtype=np.float32)

    nc = _get_program()

    shared = {
        "w1": _dense_weight(idx1, w1),
        "w2": _dense_weight(idx2, w2),
        "w3": _dense_weight(idx3, w3),
        "b1": b1,
        "b2": b2,
        "b3": b3,
        "fcwt": np.ascontiguousarray(fc_w.T),
        "fcbh": np.ascontiguousarray((fc_b / 2.0).reshape(NCLS, 1)),
        "onesd": np.ones((NCLS, 1), dtype=np.float32),
    }
    in_maps = []
    for i in range(NCORES):
        m = dict(shared)
        m["xt"] = np.ascontiguousarray(x[i * BS : (i + 1) * BS].T)
        in_maps.append(m)

    from concourse.bass_utils import run_bass_kernel_spmd

    res = run_bass_kernel_spmd(nc, in_maps, list(range(NCORES)))
    kernel._last = res

    out = np.empty((B, NCLS), dtype=np.float32)
    for i, r in enumerate(res.results):
        out[i * BS : (i + 1) * BS] = r["out_t"].T
    return out


kernel._last = None


# revision 8
# speedup vs baseline: 1.4301x; 1.4301x over previous
"""Trainium2 Bass kernel for CircularNN (3 sparse gather-layers + dense head + softmax).

Strategy:
  - Pure data parallel over batch: 65536 rows -> 8 cores x 8192.
  - Layers 2/3 (fan-in 4/8): the index tables are fixed, so each layer is
    h @ W_dense for a scatter-built dense [784,784] matrix (host builds the
    matrix as pure table preprocessing; all FLOPs run on device). Dense is
    forced here: random fan-in >=4 cannot be partition-routed on the PE.
  - Layer 1 (fan-in 2) skips the PE entirely: host pre-gathers the two
    source rows per output feature (A0 = x^T[idx1[:,0]], A1 = x^T[idx1[:,1]]
    - pure indexing, no host FLOPs), and the device computes
    gelu(w0*A0 + w1*A1 + b) on VectorE/ScalarE with per-partition scalars.
  - Activations are feature-major (transposed) on chip so the tensor engine
    contracts along partitions.
  - Matmuls run in float32r (fp32-width, ~2x faster than plain fp32 on PE).
  - GELU (exact, erf-based) fused with the per-feature bias on ScalarE.
  - Softmax with no ACT table switch: exp(l) = (1+t)/(1-t), t = tanh(l/2)
    (tanh is in the gelu LUT set); partition reduce + broadcast are tiny PE
    matmuls. The whole softmax tail is software-pipelined one chunk behind
    the matmul stream so the PE never waits on the DVE reciprocal chain.
"""

import os
import sys
import types

sys.path.insert(0, "/opt/trn_rl_repo")

import numpy as np


def _ensure_axon_hooks():
    """concourse's trace path imports antenv.axon_hooks, which this image
    lacks. Provide it (and register the NTFF profile hook when possible) so
    trace=True/BASS_TRACE=1 works instead of crashing."""
    try:
        import antenv
    except ImportError:
        return
    if "antenv.axon_hooks" in sys.modules:
        return
    hooks = types.ModuleType("antenv.axon_hooks")
    hooks._hook = None

    def set_axon_ntff_profile_hook(h):
        hooks._hook = h

    def get_axon_ntff_profile_hook():
        return hooks._hook

    hooks.set_axon_ntff_profile_hook = set_axon_ntff_profile_hook
    hooks.get_axon_ntff_profile_hook = get_axon_ntff_profile_hook
    sys.modules["antenv.axon_hooks"] = hooks
    antenv.axon_hooks = hooks
    try:
        from trn_agent_boot.trn_boot import _ntff_profile_via_ctypes

        hook = _ntff_profile_via_ctypes("/opt/axon/libaxon_pjrt.so")
        if hook is not None:
            hooks._hook = hook
    except Exception:
        pass


_ensure_axon_hooks()

B, D, NCLS = 65536, 784, 10
NCORES = 8
BS = B // NCORES  # 8192 rows per core
CHUNK = 512  # batch columns per tile pass (fp32 moving-operand max)
NCH = BS // CHUNK  # 16
P = 112  # partition tile: 784 = 7 * 112
KT = D // P  # 7 contraction/output tiles

_CACHE = {}


def _build_program():
    from concourse import bacc, mybir, tile

    f32 = mybir.dt.float32
    f32r = mybir.dt.float32r
    AF = mybir.ActivationFunctionType
    ALU = mybir.AluOpType

    nc = bacc.Bacc("TRN2", target_bir_lowering=False, debug=False)

    a0d = nc.dram_tensor("a0", [D, BS], f32, kind="ExternalInput")
    a1d = nc.dram_tensor("a1", [D, BS], f32, kind="ExternalInput")
    wd = {
        i: nc.dram_tensor(f"w{i}", [D, D], f32, kind="ExternalInput") for i in (2, 3)
    }
    bd = [nc.dram_tensor(f"b{i}", [D], f32, kind="ExternalInput") for i in (1, 2, 3)]
    w1ad = nc.dram_tensor("w1a", [D], f32, kind="ExternalInput")
    w1bd = nc.dram_tensor("w1b", [D], f32, kind="ExternalInput")
    fcw = nc.dram_tensor("fcwt", [D, NCLS], f32, kind="ExternalInput")
    fcbh = nc.dram_tensor("fcbh", [NCLS, 1], f32, kind="ExternalInput")
    onesd = nc.dram_tensor("onesd", [NCLS, 1], f32, kind="ExternalInput")
    out_t = nc.dram_tensor("out_t", [NCLS, BS], f32, kind="ExternalOutput")

    with tile.TileContext(nc) as tc:
        with (
            tc.tile_pool(name="const", bufs=1) as cpool,
            tc.tile_pool(name="ain", bufs=2) as apool,
            tc.tile_pool(name="h1b", bufs=2) as h1pool,
            tc.tile_pool(name="hbuf", bufs=1) as hpool,
            tc.tile_pool(name="eps", bufs=2) as epool,
            tc.tile_pool(name="ps", bufs=2, space="PSUM") as pspool,
            tc.tile_pool(name="psfc", bufs=2, space="PSUM") as fcpool,
            tc.tile_pool(name="pssm", bufs=2, space="PSUM") as smpool,
        ):
            # ---- persistent weights / tables ----
            ws = {}
            for i in (2, 3):
                t = cpool.tile([P, KT, D], f32r, tag=f"w{i}")
                nc.sync.dma_start(
                    out=t[:],
                    in_=wd[i][:].rearrange("(kt p) n -> p kt n", p=P).bitcast(f32r),
                )
                ws[i] = t
            bs = []
            for i in range(3):
                t = cpool.tile([P, KT], f32, tag=f"b{i}")
                nc.sync.dma_start(out=t[:], in_=bd[i][:].rearrange("(m p) -> p m", p=P))
                bs.append(t)
            w1as = cpool.tile([P, KT], f32, tag="w1a")
            nc.sync.dma_start(out=w1as[:], in_=w1ad[:].rearrange("(m p) -> p m", p=P))
            w1bs = cpool.tile([P, KT], f32, tag="w1b")
            nc.sync.dma_start(out=w1bs[:], in_=w1bd[:].rearrange("(m p) -> p m", p=P))
            fcws = cpool.tile([P, KT, NCLS], f32r, tag="fcw")
            nc.sync.dma_start(
                out=fcws[:],
                in_=fcw[:].rearrange("(kt p) n -> p kt n", p=P).bitcast(f32r),
            )
            fcbhs = cpool.tile([NCLS, 1], f32, tag="fcbh")
            nc.sync.dma_start(out=fcbhs[:], in_=fcbh[:])
            ones_k = cpool.tile([NCLS, 1], f32r, tag="ones_k")
            nc.sync.dma_start(out=ones_k[:], in_=onesd[:].bitcast(f32r))
            ones_m = cpool.tile([1, NCLS], f32r, tag="ones_m")
            nc.sync.dma_start(
                out=ones_m[:], in_=onesd[:].rearrange("n o -> o n").bitcast(f32r)
            )

            a0_r = a0d[:].rearrange("(kt p) b -> p kt b", p=P)
            a1_r = a1d[:].rearrange("(kt p) b -> p kt b", p=P)

            def emit_epilogue(state):
                """Softmax tail for a finished chunk (runs one chunk behind)."""
                psfc, c0 = state
                t_sb = epool.tile([NCLS, CHUNK], f32, tag="t")
                nc.scalar.activation(
                    t_sb[:], psfc[:], AF.Tanh, bias=fcbhs[:], scale=0.5
                )
                u = epool.tile([NCLS, CHUNK], f32, tag="u")
                nc.vector.tensor_scalar(u[:], t_sb[:], -1.0, 1.0, ALU.mult, ALU.add)
                r = epool.tile([NCLS, CHUNK], f32, tag="r")
                nc.vector.reciprocal(r[:], u[:])
                nc.vector.tensor_scalar_add(t_sb[:], t_sb[:], 1.0)
                e = epool.tile([NCLS, CHUNK], f32r, tag="e")
                with nc.allow_low_precision(reason="f32r is fp32-width for PE"):
                    nc.vector.tensor_mul(e[:], t_sb[:], r[:])
                pss = smpool.tile([1, CHUNK], f32, tag="pss")
                nc.tensor.matmul(pss[:], ones_k[:], e[:])
                rs = epool.tile([1, CHUNK], f32r, tag="rs")
                with nc.allow_low_precision(reason="f32r is fp32-width for PE"):
                    nc.vector.reciprocal(rs[:], pss[:])
                psb = smpool.tile([NCLS, CHUNK], f32, tag="psb")
                nc.tensor.matmul(psb[:], ones_m[:], rs[:])
                prob = epool.tile([NCLS, CHUNK], f32, tag="prob")
                nc.vector.tensor_mul(prob[:], e[:].bitcast(f32), psb[:])
                nc.sync.dma_start(out=out_t[:, c0 : c0 + CHUNK], in_=prob[:])

            pending = None
            for c in range(NCH):
                c0 = c * CHUNK
                a0c = apool.tile([P, KT, CHUNK], f32, tag="a0c")
                nc.sync.dma_start(out=a0c[:], in_=a0_r[:, :, c0 : c0 + CHUNK])
                a1c = apool.tile([P, KT, CHUNK], f32, tag="a1c")
                nc.sync.dma_start(out=a1c[:], in_=a1_r[:, :, c0 : c0 + CHUNK])

                # ---- layer 1 on DVE/ACT: gelu(w0*A0 + w1*A1 + b) ----
                h1 = h1pool.tile([P, KT, CHUNK], f32r, tag="h1")
                for m in range(KT):
                    tmp = epool.tile([P, CHUNK], f32, tag="l1t")
                    nc.vector.tensor_scalar_mul(
                        tmp[:], a1c[:, m, :], w1bs[:, m : m + 1]
                    )
                    pre = epool.tile([P, CHUNK], f32, tag="l1t")
                    nc.vector.scalar_tensor_tensor(
                        pre[:],
                        a0c[:, m, :],
                        w1as[:, m : m + 1],
                        tmp[:],
                        ALU.mult,
                        ALU.add,
                    )
                    nc.scalar.activation(
                        h1[:, m, :], pre[:], AF.Gelu, bias=bs[0][:, m : m + 1]
                    )

                # ---- layers 2/3 on PE ----
                src = h1
                for L in (2, 3):
                    h = hpool.tile([P, KT, CHUNK], f32r, tag=f"h{L}")
                    for m in range(KT):
                        ps = pspool.tile([P, CHUNK], f32, tag="ps")
                        for kt in range(KT):
                            nc.tensor.matmul(
                                ps[:],
                                ws[L][:, kt, m * P : (m + 1) * P],
                                src[:, kt, :],
                                start=(kt == 0),
                                stop=(kt == KT - 1),
                            )
                        nc.scalar.activation(
                            h[:, m, :], ps[:], AF.Gelu, bias=bs[L - 1][:, m : m + 1]
                        )
                    src = h

                # dense head: logits^T = fc_w @ h3^T   -> [10, CHUNK] psum
                psfc = fcpool.tile([NCLS, CHUNK], f32, tag="psfc")
                for kt in range(KT):
                    nc.tensor.matmul(
                        psfc[:],
                        fcws[:, kt, :],
                        src[:, kt, :],
                        start=(kt == 0),
                        stop=(kt == KT - 1),
                    )

                if pending is not None:
                    emit_epilogue(pending)
                pending = (psfc, c0)
            emit_epilogue(pending)

    nc.compile()
    return nc


def _get_program():
    if "nc" not in _CACHE:
        _CACHE["nc"] = _build_program()
    return _CACHE["nc"]


def _dense_weight(idx: np.ndarray, w: np.ndarray) -> np.ndarray:
    """Scatter the [Dout, k] index/weight tables into a dense [Din, Dout]."""
    wd = np.zeros((D, D), dtype=np.float32)
    cols = np.arange(D)
    for j in range(idx.shape[1]):
        np.add.at(wd, (idx[:, j], cols), w[:, j])
    return wd


def kernel(**inputs) -> np.ndarray:
    x = np.asarray(inputs["x"], dtype=np.float32)
    idx1 = np.asarray(inputs["idx1"])
    idx2 = np.asarray(inputs["idx2"])
    idx3 = np.asarray(inputs["idx3"])
    w1 = np.asarray(inputs["w1"], dtype=np.float32)
    w2 = np.asarray(inputs["w2"], dtype=np.float32)
    w3 = np.asarray(inputs["w3"], dtype=np.float32)
    b1 = np.asarray(inputs["b1"], dtype=np.float32)
    b2 = np.asarray(inputs["b2"], dtype=np.float32)
    b3 = np.asarray(inputs["b3"], dtype=np.float32)
    fc_w = np.asarray(inputs["fc_w"], dtype=np.float32)
    fc_b = np.asarray(inputs["fc_b"], dtype=np.float32)

    nc = _get_program()

    shared = {
        "w2": _dense_weight(idx2, w2),
        "w3": _dense_weight(idx3, w3),
        "b1": b1,
        "b2": b2,
        "b3": b3,
        "w1a": np.ascontiguousarray(w1[:, 0]),
        "w1b": np.ascontiguousarray(w1[:, 1]),
        "fcwt": np.ascontiguousarray(fc_w.T),
        "fcbh": np.ascontiguousarray((fc_b / 2.0).reshape(NCLS, 1)),
        "onesd": np.ones((NCLS, 1), dtype=np.float32),
    }
    in_maps = []
    for i in range(NCORES):
        m = dict(shared)
        xsT = np.ascontiguousarray(x[i * BS : (i + 1) * BS].T)
        m["a0"] = np.ascontiguousarray(xsT[idx1[:, 0], :])
        m["a1"] = np.ascontiguousarray(xsT[idx1[:, 1], :])
        in_maps.append(m)

    from concourse.bass_utils import run_bass_kernel_spmd

    res = run_bass_kernel_spmd(nc, in_maps, list(range(NCORES)))
    kernel._last = res

    out = np.empty((B, NCLS), dtype=np.float32)
    for i, r in enumerate(res.results):
        out[i * BS : (i + 1) * BS] = r["out_t"].T
    return out


kernel._last = None


# revision 11
# speedup vs baseline: 1.5912x; 1.1126x over previous
"""Trainium2 Bass kernel for CircularNN (3 sparse gather-layers + dense head + softmax).

Strategy:
  - Pure data parallel over batch: 65536 rows -> 8 cores x 8192.
  - Layers 2/3 (fan-in 4/8): the index tables are fixed, so each layer is
    h @ W_dense for a scatter-built dense [784,784] matrix (host builds the
    matrix as pure table preprocessing; all FLOPs run on device). Dense is
    forced here: random fan-in >=4 cannot be partition-routed on the PE.
  - Layer 1 (fan-in 2) skips the PE entirely: host pre-gathers the two
    source rows per output feature (A0 = x^T[idx1[:,0]], A1 = x^T[idx1[:,1]]
    - pure indexing, no host FLOPs), and the device computes
    gelu(w0*A0 + w1*A1 + b) on VectorE/ScalarE with per-partition scalars.
  - Activations are feature-major (transposed) on chip so the tensor engine
    contracts along partitions.
  - Matmuls run in float32r (fp32-width, ~2x faster than plain fp32 on PE).
  - GELU (exact, erf-based) fused with the per-feature bias on ScalarE.
  - Softmax with no ACT table switch: exp(l) = (1+t)/(1-t), t = tanh(l/2)
    (tanh is in the gelu LUT set); partition reduce + broadcast are tiny PE
    matmuls. The whole softmax tail is software-pipelined one chunk behind
    the matmul stream so the PE never waits on the DVE reciprocal chain.
"""

import os
import sys
import types

sys.path.insert(0, "/opt/trn_rl_repo")

import numpy as np


def _ensure_axon_hooks():
    """concourse's trace path imports antenv.axon_hooks, which this image
    lacks. Provide it (and register the NTFF profile hook when possible) so
    trace=True/BASS_TRACE=1 works instead of crashing."""
    try:
        import antenv
    except ImportError:
        return
    if "antenv.axon_hooks" in sys.modules:
        return
    hooks = types.ModuleType("antenv.axon_hooks")
    hooks._hook = None

    def set_axon_ntff_profile_hook(h):
        hooks._hook = h

    def get_axon_ntff_profile_hook():
        return hooks._hook

    hooks.set_axon_ntff_profile_hook = set_axon_ntff_profile_hook
    hooks.get_axon_ntff_profile_hook = get_axon_ntff_profile_hook
    sys.modules["antenv.axon_hooks"] = hooks
    antenv.axon_hooks = hooks
    try:
        from trn_agent_boot.trn_boot import _ntff_profile_via_ctypes

        hook = _ntff_profile_via_ctypes("/opt/axon/libaxon_pjrt.so")
        if hook is not None:
            hooks._hook = hook
    except Exception:
        pass


_ensure_axon_hooks()

B, D, NCLS = 65536, 784, 10
NCORES = 8
BS = B // NCORES  # 8192 rows per core
CHUNK = 512  # batch columns per tile pass (fp32 moving-operand max)
NCH = BS // CHUNK  # 16
P = 112  # partition tile: 784 = 7 * 112
KT = D // P  # 7 contraction/output tiles

_CACHE = {}


def _build_program():
    from concourse import bacc, mybir, tile

    f32 = mybir.dt.float32
    f32r = mybir.dt.float32r
    AF = mybir.ActivationFunctionType
    ALU = mybir.AluOpType

    nc = bacc.Bacc("TRN2", target_bir_lowering=False, debug=False)

    a0d = nc.dram_tensor("a0", [D, BS], f32, kind="ExternalInput")
    a1d = nc.dram_tensor("a1", [D, BS], f32, kind="ExternalInput")
    wd = {
        i: nc.dram_tensor(f"w{i}", [D, D], f32, kind="ExternalInput") for i in (2, 3)
    }
    bd = [nc.dram_tensor(f"b{i}", [D], f32, kind="ExternalInput") for i in (1, 2, 3)]
    w1ad = nc.dram_tensor("w1a", [D], f32, kind="ExternalInput")
    w1bd = nc.dram_tensor("w1b", [D], f32, kind="ExternalInput")
    fcw = nc.dram_tensor("fcwt", [D, NCLS], f32, kind="ExternalInput")
    fcbh = nc.dram_tensor("fcbh", [NCLS, 1], f32, kind="ExternalInput")
    onesd = nc.dram_tensor("onesd", [NCLS, 1], f32, kind="ExternalInput")
    out_t = nc.dram_tensor("out_t", [NCLS, BS], f32, kind="ExternalOutput")

    with tile.TileContext(nc) as tc:
        with (
            tc.tile_pool(name="const", bufs=1) as cpool,
            tc.tile_pool(name="ain", bufs=2) as apool,
            tc.tile_pool(name="h1b", bufs=2) as h1pool,
            tc.tile_pool(name="hbuf", bufs=1) as hpool,
            tc.tile_pool(name="eps", bufs=2) as epool,
            tc.tile_pool(name="ps", bufs=2, space="PSUM") as pspool,
            tc.tile_pool(name="psfc", bufs=2, space="PSUM") as fcpool,
            tc.tile_pool(name="pssm", bufs=2, space="PSUM") as smpool,
        ):
            # ---- persistent weights / tables ----
            ws = {}
            for i in (2, 3):
                t = cpool.tile([P, KT, D], f32r, tag=f"w{i}")
                nc.sync.dma_start(
                    out=t[:],
                    in_=wd[i][:].rearrange("(kt p) n -> p kt n", p=P).bitcast(f32r),
                )
                ws[i] = t
            bs = []
            for i in range(3):
                t = cpool.tile([P, KT], f32, tag=f"b{i}")
                nc.sync.dma_start(out=t[:], in_=bd[i][:].rearrange("(m p) -> p m", p=P))
                bs.append(t)
            w1as = cpool.tile([P, KT], f32, tag="w1a")
            nc.sync.dma_start(out=w1as[:], in_=w1ad[:].rearrange("(m p) -> p m", p=P))
            w1bs = cpool.tile([P, KT], f32, tag="w1b")
            nc.sync.dma_start(out=w1bs[:], in_=w1bd[:].rearrange("(m p) -> p m", p=P))
            fcws = cpool.tile([P, KT, NCLS], f32r, tag="fcw")
            nc.sync.dma_start(
                out=fcws[:],
                in_=fcw[:].rearrange("(kt p) n -> p kt n", p=P).bitcast(f32r),
            )
            fcbhs = cpool.tile([NCLS, 1], f32, tag="fcbh")
            nc.sync.dma_start(out=fcbhs[:], in_=fcbh[:])
            ones_k = cpool.tile([NCLS, 1], f32r, tag="ones_k")
            nc.sync.dma_start(out=ones_k[:], in_=onesd[:].bitcast(f32r))
            ones_m = cpool.tile([1, NCLS], f32r, tag="ones_m")
            nc.sync.dma_start(
                out=ones_m[:], in_=onesd[:].rearrange("n o -> o n").bitcast(f32r)
            )

            a0_r = a0d[:].rearrange("(kt p) b -> p kt b", p=P)
            a1_r = a1d[:].rearrange("(kt p) b -> p kt b", p=P)

            def epilogue_a(state):
                """Softmax head for chunk c-1: exp via tanh + class-sum matmul."""
                psfc, c0 = state
                t_sb = epool.tile([NCLS, CHUNK], f32, tag="t")
                nc.scalar.activation(
                    t_sb[:], psfc[:], AF.Tanh, bias=fcbhs[:], scale=0.5
                )
                u = epool.tile([NCLS, CHUNK], f32, tag="u")
                nc.vector.tensor_scalar(u[:], t_sb[:], -1.0, 1.0, ALU.mult, ALU.add)
                r = epool.tile([NCLS, CHUNK], f32, tag="r")
                nc.vector.reciprocal(r[:], u[:])
                nc.vector.tensor_scalar_add(t_sb[:], t_sb[:], 1.0)
                e = epool.tile([NCLS, CHUNK], f32r, tag="e")
                with nc.allow_low_precision(reason="f32r is fp32-width for PE"):
                    nc.vector.tensor_mul(e[:], t_sb[:], r[:])
                pss = smpool.tile([1, CHUNK], f32, tag="pss")
                nc.tensor.matmul(pss[:], ones_k[:], e[:])
                rs = epool.tile([1, CHUNK], f32r, tag="rs")
                with nc.allow_low_precision(reason="f32r is fp32-width for PE"):
                    nc.vector.reciprocal(rs[:], pss[:])
                return (e, rs, c0)

            def epilogue_b(state):
                """Softmax tail for chunk c-2: broadcast 1/sum, scale, store."""
                e, rs, c0 = state
                psb = smpool.tile([NCLS, CHUNK], f32, tag="psb")
                nc.tensor.matmul(psb[:], ones_m[:], rs[:])
                prob = epool.tile([NCLS, CHUNK], f32, tag="prob")
                nc.vector.tensor_mul(prob[:], e[:].bitcast(f32), psb[:])
                nc.sync.dma_start(out=out_t[:, c0 : c0 + CHUNK], in_=prob[:])

            pend_a = None
            pend_b = None
            for c in range(NCH):
                c0 = c * CHUNK
                a0c = apool.tile([P, KT, CHUNK], f32, tag="a0c")
                nc.sync.dma_start(out=a0c[:], in_=a0_r[:, :, c0 : c0 + CHUNK])
                a1c = apool.tile([P, KT, CHUNK], f32, tag="a1c")
                nc.sync.dma_start(out=a1c[:], in_=a1_r[:, :, c0 : c0 + CHUNK])

                # ---- layer 1 on DVE/ACT: gelu(w0*A0 + w1*A1 + b) ----
                h1 = h1pool.tile([P, KT, CHUNK], f32r, tag="h1")
                for m in range(KT):
                    tmp = epool.tile([P, CHUNK], f32, tag="l1t")
                    nc.vector.tensor_scalar_mul(
                        tmp[:], a1c[:, m, :], w1bs[:, m : m + 1]
                    )
                    pre = epool.tile([P, CHUNK], f32, tag="l1t")
                    nc.vector.scalar_tensor_tensor(
                        pre[:],
                        a0c[:, m, :],
                        w1as[:, m : m + 1],
                        tmp[:],
                        ALU.mult,
                        ALU.add,
                    )
                    nc.scalar.activation(
                        h1[:, m, :], pre[:], AF.Gelu, bias=bs[0][:, m : m + 1]
                    )

                # ---- layers 2/3 on PE ----
                src = h1
                for L in (2, 3):
                    h = hpool.tile([P, KT, CHUNK], f32r, tag=f"h{L}")
                    for m in range(KT):
                        ps = pspool.tile([P, CHUNK], f32, tag="ps")
                        for kt in range(KT):
                            nc.tensor.matmul(
                                ps[:],
                                ws[L][:, kt, m * P : (m + 1) * P],
                                src[:, kt, :],
                                start=(kt == 0),
                                stop=(kt == KT - 1),
                            )
                        nc.scalar.activation(
                            h[:, m, :], ps[:], AF.Gelu, bias=bs[L - 1][:, m : m + 1]
                        )
                    src = h

                # dense head: logits^T = fc_w @ h3^T   -> [10, CHUNK] psum
                psfc = fcpool.tile([NCLS, CHUNK], f32, tag="psfc")
                for kt in range(KT):
                    nc.tensor.matmul(
                        psfc[:],
                        fcws[:, kt, :],
                        src[:, kt, :],
                        start=(kt == 0),
                        stop=(kt == KT - 1),
                    )

                new_b = epilogue_a(pend_a) if pend_a is not None else None
                if pend_b is not None:
                    epilogue_b(pend_b)
                pend_b = new_b
                pend_a = (psfc, c0)
            new_b = epilogue_a(pend_a)
            if pend_b is not None:
                epilogue_b(pend_b)
            epilogue_b(new_b)

    nc.compile()
    return nc


def _get_program():
    if "nc" not in _CACHE:
        _CACHE["nc"] = _build_program()
    return _CACHE["nc"]


def _dense_weight(idx: np.ndarray, w: np.ndarray) -> np.ndarray:
    """Scatter the [Dout, k] index/weight tables into a dense [Din, Dout]."""
    wd = np.zeros((D, D), dtype=np.float32)
    cols = np.arange(D)
    for j in range(idx.shape[1]):
        np.add.at(wd, (idx[:, j], cols), w[:, j])
    return wd


def kernel(**inputs) -> np.ndarray:
    x = np.asarray(inputs["x"], dtype=np.float32)
    idx1 = np.asarray(inputs["idx1"])
    idx2 = np.asarray(inputs["idx2"])
    idx3 = np.asarray(inputs["idx3"])
    w1 = np.asarray(inputs["w1"], dtype=np.float32)
    w2 = np.asarray(inputs["w2"], dtype=np.float32)
    w3 = np.asarray(inputs["w3"], dtype=np.float32)
    b1 = np.asarray(inputs["b1"], dtype=np.float32)
    b2 = np.asarray(inputs["b2"], dtype=np.float32)
    b3 = np.asarray(inputs["b3"], dtype=np.float32)
    fc_w = np.asarray(inputs["fc_w"], dtype=np.float32)
    fc_b = np.asarray(inputs["fc_b"], dtype=np.float32)

    nc = _get_program()

    shared = {
        "w2": _dense_weight(idx2, w2),
        "w3": _dense_weight(idx3, w3),
        "b1": b1,
        "b2": b2,
        "b3": b3,
        "w1a": np.ascontiguousarray(w1[:, 0]),
        "w1b": np.ascontiguousarray(w1[:, 1]),
        "fcwt": np.ascontiguousarray(fc_w.T),
        "fcbh": np.ascontiguousarray((fc_b / 2.0).reshape(NCLS, 1)),
        "onesd": np.ones((NCLS, 1), dtype=np.float32),
    }
    in_maps = []
    for i in range(NCORES):
        m = dict(shared)
        xsT = np.ascontiguousarray(x[i * BS : (i + 1) * BS].T)
        m["a0"] = np.ascontiguousarray(xsT[idx1[:, 0], :])
        m["a1"] = np.ascontiguousarray(xsT[idx1[:, 1], :])
        in_maps.append(m)

    from concourse.bass_utils import run_bass_kernel_spmd

    res = run_bass_kernel_spmd(nc, in_maps, list(range(NCORES)))
    kernel._last = res

    out = np.empty((B, NCLS), dtype=np.float32)
    for i, r in enumerate(res.results):
        out[i * BS : (i + 1) * BS] = r["out_t"].T
    return out


kernel._last = None


# revision 15
# speedup vs baseline: 1.6176x; 1.0166x over previous
"""Trainium2 Bass kernel for CircularNN (3 sparse gather-layers + dense head + softmax).

Strategy:
  - Pure data parallel over batch: 65536 rows -> 8 cores x 8192.
  - Layers 2/3 (fan-in 4/8): the index tables are fixed, so each layer is
    h @ W_dense for a scatter-built dense [784,784] matrix (host builds the
    matrix as pure table preprocessing; all FLOPs run on device). Dense is
    forced here: random fan-in >=4 cannot be partition-routed on the PE.
  - Layer 1 (fan-in 2) skips the PE entirely: host pre-gathers the two
    source rows per output feature (A0 = x^T[idx1[:,0]], A1 = x^T[idx1[:,1]]
    - pure indexing, no host FLOPs), and the device computes
    gelu(w0*A0 + w1*A1 + b) on VectorE/ScalarE with per-partition scalars.
  - Activations are feature-major (transposed) on chip so the tensor engine
    contracts along partitions.
  - Matmuls run in float32r (fp32-width, ~2x faster than plain fp32 on PE).
  - GELU (exact, erf-based) fused with the per-feature bias on ScalarE.
  - Softmax with no ACT table switch: exp(l) = (1+t)/(1-t), t = tanh(l/2)
    (tanh is in the gelu LUT set); partition reduce + broadcast are tiny PE
    matmuls. The whole softmax tail is software-pipelined one chunk behind
    the matmul stream so the PE never waits on the DVE reciprocal chain.
"""

import os
import sys
import types

sys.path.insert(0, "/opt/trn_rl_repo")

import numpy as np


def _ensure_axon_hooks():
    """concourse's trace path imports antenv.axon_hooks, which this image
    lacks. Provide it (and register the NTFF profile hook when possible) so
    trace=True/BASS_TRACE=1 works instead of crashing."""
    try:
        import antenv
    except ImportError:
        return
    if "antenv.axon_hooks" in sys.modules:
        return
    hooks = types.ModuleType("antenv.axon_hooks")
    hooks._hook = None

    def set_axon_ntff_profile_hook(h):
        hooks._hook = h

    def get_axon_ntff_profile_hook():
        return hooks._hook

    hooks.set_axon_ntff_profile_hook = set_axon_ntff_profile_hook
    hooks.get_axon_ntff_profile_hook = get_axon_ntff_profile_hook
    sys.modules["antenv.axon_hooks"] = hooks
    antenv.axon_hooks = hooks
    try:
        from trn_agent_boot.trn_boot import _ntff_profile_via_ctypes

        hook = _ntff_profile_via_ctypes("/opt/axon/libaxon_pjrt.so")
        if hook is not None:
            hooks._hook = hook
    except Exception:
        pass


_ensure_axon_hooks()

B, D, NCLS = 65536, 784, 10
NCORES = 8
BS = B // NCORES  # 8192 rows per core
CHUNK = 512  # batch columns per tile pass (fp32 moving-operand max)
NCH = BS // CHUNK  # 16
P = 112  # partition tile: 784 = 7 * 112
KT = D // P  # 7 contraction/output tiles

_CACHE = {}


def _build_program():
    from concourse import bacc, mybir, tile

    f32 = mybir.dt.float32
    f32r = mybir.dt.float32r
    AF = mybir.ActivationFunctionType
    ALU = mybir.AluOpType

    nc = bacc.Bacc("TRN2", target_bir_lowering=False, debug=False)

    a0d = nc.dram_tensor("a0", [D, BS], f32, kind="ExternalInput")
    a1d = nc.dram_tensor("a1", [D, BS], f32, kind="ExternalInput")
    wd = {
        i: nc.dram_tensor(f"w{i}", [D, D], f32, kind="ExternalInput") for i in (2, 3)
    }
    bd = [nc.dram_tensor(f"b{i}", [D], f32, kind="ExternalInput") for i in (1, 2, 3)]
    w1ad = nc.dram_tensor("w1a", [D], f32, kind="ExternalInput")
    w1bd = nc.dram_tensor("w1b", [D], f32, kind="ExternalInput")
    fcw = nc.dram_tensor("fcwt", [D, NCLS], f32, kind="ExternalInput")
    fcbh = nc.dram_tensor("fcbh", [NCLS, 1], f32, kind="ExternalInput")
    onesd = nc.dram_tensor("onesd", [NCLS, 1], f32, kind="ExternalInput")
    out_t = nc.dram_tensor("out_t", [NCLS, BS], f32, kind="ExternalOutput")

    with tile.TileContext(nc) as tc:
        with (
            tc.tile_pool(name="const", bufs=1) as cpool,
            tc.tile_pool(name="ain", bufs=2) as apool,
            tc.tile_pool(name="h1b", bufs=2) as h1pool,
            tc.tile_pool(name="hbuf", bufs=1) as hpool,
            tc.tile_pool(name="eps", bufs=2) as epool,
            tc.tile_pool(name="ps", bufs=3, space="PSUM") as pspool,
            tc.tile_pool(name="psfc", bufs=2, space="PSUM") as fcpool,
            tc.tile_pool(name="pssm", bufs=1, space="PSUM") as smpool,
        ):
            # ---- persistent weights / tables ----
            # (emitted small-tables + chunk-0/1 inputs first so layer 1 can
            # start while the big weight matrices stream in behind them)
            bs = []
            for i in range(3):
                t = cpool.tile([P, KT], f32, tag=f"b{i}")
                nc.sync.dma_start(out=t[:], in_=bd[i][:].rearrange("(m p) -> p m", p=P))
                bs.append(t)
            w1as = cpool.tile([P, KT], f32, tag="w1a")
            nc.sync.dma_start(out=w1as[:], in_=w1ad[:].rearrange("(m p) -> p m", p=P))
            w1bs = cpool.tile([P, KT], f32, tag="w1b")
            nc.sync.dma_start(out=w1bs[:], in_=w1bd[:].rearrange("(m p) -> p m", p=P))
            fcws = cpool.tile([P, KT, NCLS], f32r, tag="fcw")
            nc.sync.dma_start(
                out=fcws[:],
                in_=fcw[:].rearrange("(kt p) n -> p kt n", p=P).bitcast(f32r),
            )
            fcbhs = cpool.tile([NCLS, 1], f32, tag="fcbh")
            nc.sync.dma_start(out=fcbhs[:], in_=fcbh[:])
            ones_k = cpool.tile([NCLS, 1], f32r, tag="ones_k")
            nc.sync.dma_start(out=ones_k[:], in_=onesd[:].bitcast(f32r))
            ones_m = cpool.tile([1, NCLS], f32r, tag="ones_m")
            nc.sync.dma_start(
                out=ones_m[:], in_=onesd[:].rearrange("n o -> o n").bitcast(f32r)
            )

            a0_r = a0d[:].rearrange("(kt p) b -> p kt b", p=P)
            a1_r = a1d[:].rearrange("(kt p) b -> p kt b", p=P)

            def load_a(c):
                c0 = c * CHUNK
                a0c = apool.tile([P, KT, CHUNK], f32, tag="a0c")
                nc.sync.dma_start(out=a0c[:], in_=a0_r[:, :, c0 : c0 + CHUNK])
                a1c = apool.tile([P, KT, CHUNK], f32, tag="a1c")
                nc.sync.dma_start(out=a1c[:], in_=a1_r[:, :, c0 : c0 + CHUNK])
                return a0c, a1c

            prefetched = {c: load_a(c) for c in (0, 1)}

            # big weight matrices, split per output-column block so the first
            # L2 matmuls only wait for the first slice
            ws = {}
            for i in (2, 3):
                t = cpool.tile([P, KT, D], f32r, tag=f"w{i}")
                w_r = wd[i][:].rearrange("(kt p) n -> p kt n", p=P).bitcast(f32r)
                for m in range(KT):
                    nc.sync.dma_start(
                        out=t[:, :, m * P : (m + 1) * P],
                        in_=w_r[:, :, m * P : (m + 1) * P],
                    )
                ws[i] = t

            def epilogue_a(state):
                """Softmax head for chunk c-1: exp via tanh + class-sum matmul."""
                psfc, c0 = state
                t_sb = epool.tile([NCLS, CHUNK], f32, tag="t")
                nc.scalar.activation(
                    t_sb[:], psfc[:], AF.Tanh, bias=fcbhs[:], scale=0.5
                )
                u = epool.tile([NCLS, CHUNK], f32, tag="u")
                nc.vector.tensor_scalar(u[:], t_sb[:], -1.0, 1.0, ALU.mult, ALU.add)
                r = epool.tile([NCLS, CHUNK], f32, tag="r")
                nc.vector.reciprocal(r[:], u[:])
                nc.vector.tensor_scalar_add(t_sb[:], t_sb[:], 1.0)
                e = epool.tile([NCLS, CHUNK], f32r, tag="e")
                with nc.allow_low_precision(reason="f32r is fp32-width for PE"):
                    nc.vector.tensor_mul(e[:], t_sb[:], r[:])
                pss = smpool.tile([1, CHUNK], f32, tag="pss")
                nc.tensor.matmul(pss[:], ones_k[:], e[:])
                rs = epool.tile([1, CHUNK], f32r, tag="rs")
                with nc.allow_low_precision(reason="f32r is fp32-width for PE"):
                    nc.vector.reciprocal(rs[:], pss[:])
                return (e, rs, c0)

            def epilogue_b(state):
                """Softmax tail for chunk c-2: broadcast 1/sum, scale, store."""
                e, rs, c0 = state
                psb = smpool.tile([NCLS, CHUNK], f32, tag="psb")
                nc.tensor.matmul(psb[:], ones_m[:], rs[:])
                prob = epool.tile([NCLS, CHUNK], f32, tag="prob")
                nc.vector.tensor_mul(prob[:], e[:].bitcast(f32), psb[:])
                nc.sync.dma_start(out=out_t[:, c0 : c0 + CHUNK], in_=prob[:])

            pend_a = None
            pend_b = None
            for c in range(NCH):
                c0 = c * CHUNK
                if c in prefetched:
                    a0c, a1c = prefetched.pop(c)
                else:
                    a0c, a1c = load_a(c)

                # ---- layer 1 on DVE/ACT: gelu(w0*A0 + w1*A1 + b) ----
                h1 = h1pool.tile([P, KT, CHUNK], f32r, tag="h1")
                for m in range(KT):
                    tmp = epool.tile([P, CHUNK], f32, tag="l1t")
                    nc.vector.tensor_scalar_mul(
                        tmp[:], a1c[:, m, :], w1bs[:, m : m + 1]
                    )
                    pre = epool.tile([P, CHUNK], f32, tag="l1t")
                    nc.vector.scalar_tensor_tensor(
                        pre[:],
                        a0c[:, m, :],
                        w1as[:, m : m + 1],
                        tmp[:],
                        ALU.mult,
                        ALU.add,
                    )
                    nc.scalar.activation(
                        h1[:, m, :], pre[:], AF.Gelu, bias=bs[0][:, m : m + 1]
                    )

                # ---- layers 2/3 on PE ----
                src = h1
                for L in (2, 3):
                    h = hpool.tile([P, KT, CHUNK], f32r, tag=f"h{L}")
                    for m in range(KT):
                        ps = pspool.tile([P, CHUNK], f32, tag="ps")
                        for kt in range(KT):
                            nc.tensor.matmul(
                                ps[:],
                                ws[L][:, kt, m * P : (m + 1) * P],
                                src[:, kt, :],
                                start=(kt == 0),
                                stop=(kt == KT - 1),
                            )
                        nc.scalar.activation(
                            h[:, m, :], ps[:], AF.Gelu, bias=bs[L - 1][:, m : m + 1]
                        )
                    src = h

                # dense head: logits^T = fc_w @ h3^T   -> [10, CHUNK] psum
                psfc = fcpool.tile([NCLS, CHUNK], f32, tag="psfc")
                for kt in range(KT):
                    nc.tensor.matmul(
                        psfc[:],
                        fcws[:, kt, :],
                        src[:, kt, :],
                        start=(kt == 0),
                        stop=(kt == KT - 1),
                    )

                new_b = epilogue_a(pend_a) if pend_a is not None else None
                if pend_b is not None:
                    epilogue_b(pend_b)
                pend_b = new_b
                pend_a = (psfc, c0)
            new_b = epilogue_a(pend_a)
            if pend_b is not None:
                epilogue_b(pend_b)
            epilogue_b(new_b)

    nc.compile()
    return nc


def _get_program():
    if "nc" not in _CACHE:
        _CACHE["nc"] = _build_program()
    return _CACHE["nc"]


def _dense_weight(idx: np.ndarray, w: np.ndarray) -> np.ndarray:
    """Scatter the [Dout, k] index/weight tables into a dense [Din, Dout]."""
    wd = np.zeros((D, D), dtype=np.float32)
    cols = np.arange(D)
    for j in range(idx.shape[1]):
        np.add.at(wd, (idx[:, j], cols), w[:, j])
    return wd


def kernel(**inputs) -> np.ndarray:
    x = np.asarray(inputs["x"], dtype=np.float32)
    idx1 = np.asarray(inputs["idx1"])
    idx2 = np.asarray(inputs["idx2"])
    idx3 = np.asarray(inputs["idx3"])
    w1 = np.asarray(inputs["w1"], dtype=np.float32)
    w2 = np.asarray(inputs["w2"], dtype=np.float32)
    w3 = np.asarray(inputs["w3"], dtype=np.float32)
    b1 = np.asarray(inputs["b1"], dtype=np.float32)
    b2 = np.asarray(inputs["b2"], dtype=np.float32)
    b3 = np.asarray(inputs["b3"], dtype=np.float32)
    fc_w = np.asarray(inputs["fc_w"], dtype=np.float32)
    fc_b = np.asarray(inputs["fc_b"], dtype=np.float32)

    nc = _get_program()

    shared = {
        "w2": _dense_weight(idx2, w2),
        "w3": _dense_weight(idx3, w3),
        "b1": b1,
        "b2": b2,
        "b3": b3,
        "w1a": np.ascontiguousarray(w1[:, 0]),
        "w1b": np.ascontiguousarray(w1[:, 1]),
        "fcwt": np.ascontiguousarray(fc_w.T),
        "fcbh": np.ascontiguousarray((fc_b / 2.0).reshape(NCLS, 1)),
        "onesd": np.ones((NCLS, 1), dtype=np.float32),
    }
    in_maps = []
    for i in range(NCORES):
        m = dict(shared)
        xsT = np.ascontiguousarray(x[i * BS : (i + 1) * BS].T)
        m["a0"] = np.ascontiguousarray(xsT[idx1[:, 0], :])
        m["a1"] = np.ascontiguousarray(xsT[idx1[:, 1], :])
        in_maps.append(m)

    from concourse.bass_utils import run_bass_kernel_spmd

    res = run_bass_kernel_spmd(nc, in_maps, list(range(NCORES)))
    kernel._last = res

    out = np.empty((B, NCLS), dtype=np.float32)
    for i, r in enumerate(res.results):
        out[i * BS : (i + 1) * BS] = r["out_t"].T
    return out


kernel._last = None


# revision 17
# speedup vs baseline: 1.6359x; 1.0113x over previous
"""Trainium2 Bass kernel for CircularNN (3 sparse gather-layers + dense head + softmax).

Strategy:
  - Pure data parallel over batch: 65536 rows -> 8 cores x 8192.
  - Layers 2/3 (fan-in 4/8): the index tables are fixed, so each layer is
    h @ W_dense for a scatter-built dense [784,784] matrix (host builds the
    matrix as pure table preprocessing; all FLOPs run on device). Dense is
    forced here: random fan-in >=4 cannot be partition-routed on the PE.
  - Layer 1 (fan-in 2) skips the PE entirely: host pre-gathers the two
    source rows per output feature (A0 = x^T[idx1[:,0]], A1 = x^T[idx1[:,1]]
    - pure indexing, no host FLOPs), and the device computes
    gelu(w0*A0 + w1*A1 + b) on VectorE/ScalarE with per-partition scalars.
  - Activations are feature-major (transposed) on chip so the tensor engine
    contracts along partitions.
  - Matmuls run in float32r (fp32-width, ~2x faster than plain fp32 on PE).
  - GELU (exact, erf-based) fused with the per-feature bias on ScalarE.
  - Softmax with no ACT table switch: exp(l) = (1+t)/(1-t), t = tanh(l/2)
    (tanh is in the gelu LUT set); partition reduce + broadcast are tiny PE
    matmuls. The whole softmax tail is software-pipelined one chunk behind
    the matmul stream so the PE never waits on the DVE reciprocal chain.
"""

import os
import sys
import types

sys.path.insert(0, "/opt/trn_rl_repo")

import numpy as np


def _ensure_axon_hooks():
    """concourse's trace path imports antenv.axon_hooks, which this image
    lacks. Provide it (and register the NTFF profile hook when possible) so
    trace=True/BASS_TRACE=1 works instead of crashing."""
    try:
        import antenv
    except ImportError:
        return
    if "antenv.axon_hooks" in sys.modules:
        return
    hooks = types.ModuleType("antenv.axon_hooks")
    hooks._hook = None

    def set_axon_ntff_profile_hook(h):
        hooks._hook = h

    def get_axon_ntff_profile_hook():
        return hooks._hook

    hooks.set_axon_ntff_profile_hook = set_axon_ntff_profile_hook
    hooks.get_axon_ntff_profile_hook = get_axon_ntff_profile_hook
    sys.modules["antenv.axon_hooks"] = hooks
    antenv.axon_hooks = hooks
    try:
        from trn_agent_boot.trn_boot import _ntff_profile_via_ctypes

        hook = _ntff_profile_via_ctypes("/opt/axon/libaxon_pjrt.so")
        if hook is not None:
            hooks._hook = hook
    except Exception:
        pass


_ensure_axon_hooks()

B, D, NCLS = 65536, 784, 10
NCORES = 8
BS = B // NCORES  # 8192 rows per core
CHUNK = 512  # batch columns per tile pass (fp32 moving-operand max)
NCH = BS // CHUNK  # 16
P = 112  # partition tile: 784 = 7 * 112
KT = D // P  # 7 contraction/output tiles

_CACHE = {}


def _build_program():
    from concourse import bacc, mybir, tile

    f32 = mybir.dt.float32
    f32r = mybir.dt.float32r
    AF = mybir.ActivationFunctionType
    ALU = mybir.AluOpType

    nc = bacc.Bacc("TRN2", target_bir_lowering=False, debug=False)

    # all tensors are host-packed so every DMA is [112 partitions x one
    # contiguous run per partition] (112 fat descriptors, cheap HWDGE gen)
    a01d = nc.dram_tensor("a01", [P, NCH, 2, KT, CHUNK], f32, kind="ExternalInput")
    wd = {
        i: nc.dram_tensor(f"w{i}", [P, KT, KT, P], f32, kind="ExternalInput")
        for i in (2, 3)
    }
    tabd = nc.dram_tensor("tab", [P, KT, 5], f32, kind="ExternalInput")
    fcwp = nc.dram_tensor("fcwp", [P, KT, NCLS], f32, kind="ExternalInput")
    fcbh = nc.dram_tensor("fcbh", [NCLS, 1], f32, kind="ExternalInput")
    onesd = nc.dram_tensor("onesd", [NCLS, 1], f32, kind="ExternalInput")
    out_t = nc.dram_tensor("out_t", [NCLS, BS], f32, kind="ExternalOutput")

    with tile.TileContext(nc) as tc:
        with (
            tc.tile_pool(name="const", bufs=1) as cpool,
            tc.tile_pool(name="ain", bufs=2) as apool,
            tc.tile_pool(name="h1b", bufs=2) as h1pool,
            tc.tile_pool(name="hbuf", bufs=1) as hpool,
            tc.tile_pool(name="eps", bufs=2) as epool,
            tc.tile_pool(name="ps", bufs=3, space="PSUM") as pspool,
            tc.tile_pool(name="psfc", bufs=2, space="PSUM") as fcpool,
            tc.tile_pool(name="pssm", bufs=1, space="PSUM") as smpool,
        ):
            # ---- persistent weights / tables ----
            # (emitted small-tables + chunk-0/1 inputs first so layer 1 can
            # start while the big weight matrices stream in behind them)
            tabs = cpool.tile([P, KT, 5], f32, tag="tab")
            nc.sync.dma_start(out=tabs[:], in_=tabd[:])
            bs = [tabs[:, :, i] for i in range(3)]
            w1as = tabs[:, :, 3]
            w1bs = tabs[:, :, 4]
            fcws = cpool.tile([P, KT, NCLS], f32r, tag="fcw")
            nc.sync.dma_start(out=fcws[:], in_=fcwp[:].bitcast(f32r))
            fcbhs = cpool.tile([NCLS, 1], f32, tag="fcbh")
            nc.sync.dma_start(out=fcbhs[:], in_=fcbh[:])
            ones_k = cpool.tile([NCLS, 1], f32r, tag="ones_k")
            nc.sync.dma_start(out=ones_k[:], in_=onesd[:].bitcast(f32r))
            ones_m = cpool.tile([1, NCLS], f32r, tag="ones_m")
            nc.sync.dma_start(
                out=ones_m[:], in_=onesd[:].rearrange("n o -> o n").bitcast(f32r)
            )

            def load_a(c):
                ac = apool.tile([P, 2, KT, CHUNK], f32, tag="ac")
                nc.sync.dma_start(out=ac[:], in_=a01d[:, c])
                return ac

            prefetched = {c: load_a(c) for c in (0, 1)}

            # big weight matrices, split per output-column block so the first
            # L2 matmuls only wait for the first slice
            ws = {}
            for i in (2, 3):
                t = cpool.tile([P, KT, KT, P], f32r, tag=f"w{i}")
                for m in range(KT):
                    nc.sync.dma_start(
                        out=t[:, m], in_=wd[i][:, m].bitcast(f32r)
                    )
                ws[i] = t

            def epilogue_a(state):
                """Softmax head for chunk c-1: exp via tanh + class-sum matmul."""
                psfc, c0 = state
                t_sb = epool.tile([NCLS, CHUNK], f32, tag="t")
                nc.scalar.activation(
                    t_sb[:], psfc[:], AF.Tanh, bias=fcbhs[:], scale=0.5
                )
                u = epool.tile([NCLS, CHUNK], f32, tag="u")
                nc.vector.tensor_scalar(u[:], t_sb[:], -1.0, 1.0, ALU.mult, ALU.add)
                r = epool.tile([NCLS, CHUNK], f32, tag="r")
                nc.vector.reciprocal(r[:], u[:])
                nc.vector.tensor_scalar_add(t_sb[:], t_sb[:], 1.0)
                e = epool.tile([NCLS, CHUNK], f32r, tag="e")
                with nc.allow_low_precision(reason="f32r is fp32-width for PE"):
                    nc.vector.tensor_mul(e[:], t_sb[:], r[:])
                pss = smpool.tile([1, CHUNK], f32, tag="pss")
                nc.tensor.matmul(pss[:], ones_k[:], e[:])
                rs = epool.tile([1, CHUNK], f32r, tag="rs")
                with nc.allow_low_precision(reason="f32r is fp32-width for PE"):
                    nc.vector.reciprocal(rs[:], pss[:])
                return (e, rs, c0)

            def epilogue_b(state):
                """Softmax tail for chunk c-2: broadcast 1/sum, scale, store."""
                e, rs, c0 = state
                psb = smpool.tile([NCLS, CHUNK], f32, tag="psb")
                nc.tensor.matmul(psb[:], ones_m[:], rs[:])
                prob = epool.tile([NCLS, CHUNK], f32, tag="prob")
                nc.vector.tensor_mul(prob[:], e[:].bitcast(f32), psb[:])
                nc.sync.dma_start(out=out_t[:, c0 : c0 + CHUNK], in_=prob[:])

            pend_a = None
            pend_b = None
            for c in range(NCH):
                c0 = c * CHUNK
                ac = prefetched.pop(c) if c in prefetched else load_a(c)

                # ---- layer 1 on DVE/ACT: gelu(w0*A0 + w1*A1 + b) ----
                h1 = h1pool.tile([P, KT, CHUNK], f32r, tag="h1")
                for m in range(KT):
                    tmp = epool.tile([P, CHUNK], f32, tag="l1t")
                    nc.vector.tensor_scalar_mul(
                        tmp[:], ac[:, 1, m, :], w1bs[:, m : m + 1]
                    )
                    pre = epool.tile([P, CHUNK], f32, tag="l1t")
                    nc.vector.scalar_tensor_tensor(
                        pre[:],
                        ac[:, 0, m, :],
                        w1as[:, m : m + 1],
                        tmp[:],
                        ALU.mult,
                        ALU.add,
                    )
                    nc.scalar.activation(
                        h1[:, m, :], pre[:], AF.Gelu, bias=bs[0][:, m : m + 1]
                    )

                # ---- layers 2/3 on PE ----
                src = h1
                for L in (2, 3):
                    h = hpool.tile([P, KT, CHUNK], f32r, tag=f"h{L}")
                    for m in range(KT):
                        ps = pspool.tile([P, CHUNK], f32, tag="ps")
                        for kt in range(KT):
                            nc.tensor.matmul(
                                ps[:],
                                ws[L][:, m, kt, :],
                                src[:, kt, :],
                                start=(kt == 0),
                                stop=(kt == KT - 1),
                            )
                        nc.scalar.activation(
                            h[:, m, :], ps[:], AF.Gelu, bias=bs[L - 1][:, m : m + 1]
                        )
                    src = h

                # dense head: logits^T = fc_w @ h3^T   -> [10, CHUNK] psum
                psfc = fcpool.tile([NCLS, CHUNK], f32, tag="psfc")
                for kt in range(KT):
                    nc.tensor.matmul(
                        psfc[:],
                        fcws[:, kt, :],
                        src[:, kt, :],
                        start=(kt == 0),
                        stop=(kt == KT - 1),
                    )

                new_b = epilogue_a(pend_a) if pend_a is not None else None
                if pend_b is not None:
                    epilogue_b(pend_b)
                pend_b = new_b
                pend_a = (psfc, c0)
            new_b = epilogue_a(pend_a)
            if pend_b is not None:
                epilogue_b(pend_b)
            epilogue_b(new_b)

    nc.compile()
    return nc


def _get_program():
    if "nc" not in _CACHE:
        _CACHE["nc"] = _build_program()
    return _CACHE["nc"]


def _dense_weight(idx: np.ndarray, w: np.ndarray) -> np.ndarray:
    """Scatter the [Dout, k] index/weight tables into a dense [Din, Dout]."""
    wd = np.zeros((D, D), dtype=np.float32)
    cols = np.arange(D)
    for j in range(idx.shape[1]):
        np.add.at(wd, (idx[:, j], cols), w[:, j])
    return wd


def kernel(**inputs) -> np.ndarray:
    x = np.asarray(inputs["x"], dtype=np.float32)
    idx1 = np.asarray(inputs["idx1"])
    idx2 = np.asarray(inputs["idx2"])
    idx3 = np.asarray(inputs["idx3"])
    w1 = np.asarray(inputs["w1"], dtype=np.float32)
    w2 = np.asarray(inputs["w2"], dtype=np.float32)
    w3 = np.asarray(inputs["w3"], dtype=np.float32)
    b1 = np.asarray(inputs["b1"], dtype=np.float32)
    b2 = np.asarray(inputs["b2"], dtype=np.float32)
    b3 = np.asarray(inputs["b3"], dtype=np.float32)
    fc_w = np.asarray(inputs["fc_w"], dtype=np.float32)
    fc_b = np.asarray(inputs["fc_b"], dtype=np.float32)

    nc = _get_program()

    def pack_w(wdense):
        # W[kt*P+p, m*P+n] -> [p, m, kt, n]
        return np.ascontiguousarray(
            wdense.reshape(KT, P, KT, P).transpose(1, 2, 0, 3)
        )

    def pack_vec(v):
        # v[m*P+p] -> [p, m]
        return v.reshape(KT, P).T

    tab = np.ascontiguousarray(
        np.stack(
            [pack_vec(b1), pack_vec(b2), pack_vec(b3), pack_vec(w1[:, 0]), pack_vec(w1[:, 1])],
            axis=2,
        )
    )
    shared = {
        "w2": pack_w(_dense_weight(idx2, w2)),
        "w3": pack_w(_dense_weight(idx3, w3)),
        "tab": tab,
        "fcwp": np.ascontiguousarray(
            fc_w.T.reshape(KT, P, NCLS).transpose(1, 0, 2)
        ),
        "fcbh": np.ascontiguousarray((fc_b / 2.0).reshape(NCLS, 1)),
        "onesd": np.ones((NCLS, 1), dtype=np.float32),
    }
    in_maps = []
    for i in range(NCORES):
        m = dict(shared)
        xsT = x[i * BS : (i + 1) * BS].T
        # a_j[kt*P+p, c*CHUNK+f] -> [p, c, j, kt, f]
        a = np.stack(
            [
                xsT[idx1[:, 0], :].reshape(KT, P, NCH, CHUNK).transpose(1, 2, 0, 3),
                xsT[idx1[:, 1], :].reshape(KT, P, NCH, CHUNK).transpose(1, 2, 0, 3),
            ],
            axis=2,
        )
        m["a01"] = np.ascontiguousarray(a)
        in_maps.append(m)

    from concourse.bass_utils import run_bass_kernel_spmd

    res = run_bass_kernel_spmd(nc, in_maps, list(range(NCORES)))
    kernel._last = res

    out = np.empty((B, NCLS), dtype=np.float32)
    for i, r in enumerate(res.results):
        out[i * BS : (i + 1) * BS] = r["out_t"].T
    return out


kernel._last = None


# revision 19
# speedup vs baseline: 1.6419x; 1.0036x over previous
"""Trainium2 Bass kernel for CircularNN (3 sparse gather-layers + dense head + softmax).

Strategy:
  - Pure data parallel over batch: 65536 rows -> 8 cores x 8192.
  - Layers 2/3 (fan-in 4/8): the index tables are fixed, so each layer is
    h @ W_dense for a scatter-built dense [784,784] matrix (host builds the
    matrix as pure table preprocessing; all FLOPs run on device). Dense is
    forced here: random fan-in >=4 cannot be partition-routed on the PE.
  - Layer 1 (fan-in 2) skips the PE entirely: host pre-gathers the two
    source rows per output feature (A0 = x^T[idx1[:,0]], A1 = x^T[idx1[:,1]]
    - pure indexing, no host FLOPs), and the device computes
    gelu(w0*A0 + w1*A1 + b) on VectorE/ScalarE with per-partition scalars.
  - Activations are feature-major (transposed) on chip so the tensor engine
    contracts along partitions.
  - Matmuls run in float32r (fp32-width, ~2x faster than plain fp32 on PE).
  - GELU (exact, erf-based) fused with the per-feature bias on ScalarE.
  - Softmax with no ACT table switch: exp(l) = (1+t)/(1-t), t = tanh(l/2)
    (tanh is in the gelu LUT set); partition reduce + broadcast are tiny PE
    matmuls. The whole softmax tail is software-pipelined one chunk behind
    the matmul stream so the PE never waits on the DVE reciprocal chain.
"""

import os
import sys
import types

sys.path.insert(0, "/opt/trn_rl_repo")

import numpy as np


def _ensure_axon_hooks():
    """concourse's trace path imports antenv.axon_hooks, which this image
    lacks. Provide it (and register the NTFF profile hook when possible) so
    trace=True/BASS_TRACE=1 works instead of crashing."""
    try:
        import antenv
    except ImportError:
        return
    if "antenv.axon_hooks" in sys.modules:
        return
    hooks = types.ModuleType("antenv.axon_hooks")
    hooks._hook = None

    def set_axon_ntff_profile_hook(h):
        hooks._hook = h

    def get_axon_ntff_profile_hook():
        return hooks._hook

    hooks.set_axon_ntff_profile_hook = set_axon_ntff_profile_hook
    hooks.get_axon_ntff_profile_hook = get_axon_ntff_profile_hook
    sys.modules["antenv.axon_hooks"] = hooks
    antenv.axon_hooks = hooks
    try:
        from trn_agent_boot.trn_boot import _ntff_profile_via_ctypes

        hook = _ntff_profile_via_ctypes("/opt/axon/libaxon_pjrt.so")
        if hook is not None:
            hooks._hook = hook
    except Exception:
        pass


_ensure_axon_hooks()

B, D, NCLS = 65536, 784, 10
NCORES = 8
BS = B // NCORES  # 8192 rows per core
CHUNK = 512  # batch columns per tile pass (fp32 moving-operand max)
NCH = BS // CHUNK  # 16
P = 112  # partition tile: 784 = 7 * 112
KT = D // P  # 7 contraction/output tiles

_CACHE = {}


def _build_program():
    from concourse import bacc, mybir, tile

    f32 = mybir.dt.float32
    f32r = mybir.dt.float32r
    AF = mybir.ActivationFunctionType
    ALU = mybir.AluOpType

    nc = bacc.Bacc("TRN2", target_bir_lowering=False, debug=False)

    # all tensors are host-packed so every DMA is [112 partitions x one
    # contiguous run per partition] (112 fat descriptors, cheap HWDGE gen)
    a01d = nc.dram_tensor("a01", [P, NCH, 2, KT, CHUNK], f32, kind="ExternalInput")
    wd = {
        i: nc.dram_tensor(f"w{i}", [P, KT, KT, P], f32, kind="ExternalInput")
        for i in (2, 3)
    }
    tabd = nc.dram_tensor("tab", [P, KT, 5], f32, kind="ExternalInput")
    fcwp = nc.dram_tensor("fcwp", [P, KT, NCLS], f32, kind="ExternalInput")
    fcbh = nc.dram_tensor("fcbh", [NCLS, 1], f32, kind="ExternalInput")
    onesd = nc.dram_tensor("onesd", [NCLS, 1], f32, kind="ExternalInput")
    out_t = nc.dram_tensor("out_t", [NCLS, BS], f32, kind="ExternalOutput")

    with tile.TileContext(nc) as tc:
        with (
            tc.tile_pool(name="const", bufs=1) as cpool,
            tc.tile_pool(name="ain", bufs=2) as apool,
            tc.tile_pool(name="h1b", bufs=2) as h1pool,
            tc.tile_pool(name="hbuf", bufs=1) as hpool,
            tc.tile_pool(name="eps", bufs=2) as epool,
            tc.tile_pool(name="ps", bufs=3, space="PSUM") as pspool,
            tc.tile_pool(name="psfc", bufs=2, space="PSUM") as fcpool,
            tc.tile_pool(name="pssm", bufs=1, space="PSUM") as smpool,
        ):
            # ---- persistent weights / tables ----
            # (emitted small-tables + chunk-0/1 inputs first so layer 1 can
            # start while the big weight matrices stream in behind them)
            # DMA issue order is tuned for the pipeline head: the L1 tables
            # and chunk-0 inputs first (chunk 0 per m-slice so layer 1 starts
            # on the first slice), then W2 (needed by the first matmuls),
            # then the chunk-1 prefetch and everything else.
            tabs = cpool.tile([P, KT, 5], f32, tag="tab")
            nc.sync.dma_start(out=tabs[:], in_=tabd[:])
            bs = [tabs[:, :, i] for i in range(3)]
            w1as = tabs[:, :, 3]
            w1bs = tabs[:, :, 4]

            def load_a(c, split=False):
                ac = apool.tile([P, 2, KT, CHUNK], f32, tag="ac")
                if split:
                    for m in range(KT):
                        nc.sync.dma_start(
                            out=ac[:, :, m, :], in_=a01d[:, c, :, m, :]
                        )
                else:
                    nc.sync.dma_start(out=ac[:], in_=a01d[:, c])
                return ac

            prefetched = {0: load_a(0, split=True)}

            ws = {}
            for i in (2, 3):
                ws[i] = cpool.tile([P, KT, KT, P], f32r, tag=f"w{i}", name=f"w{i}s")
            for m in range(KT):
                nc.sync.dma_start(out=ws[2][:, m], in_=wd[2][:, m].bitcast(f32r))

            prefetched[1] = load_a(1)

            for m in range(KT):
                nc.sync.dma_start(out=ws[3][:, m], in_=wd[3][:, m].bitcast(f32r))

            fcws = cpool.tile([P, KT, NCLS], f32r, tag="fcw")
            nc.sync.dma_start(out=fcws[:], in_=fcwp[:].bitcast(f32r))
            fcbhs = cpool.tile([NCLS, 1], f32, tag="fcbh")
            nc.sync.dma_start(out=fcbhs[:], in_=fcbh[:])
            ones_k = cpool.tile([NCLS, 1], f32r, tag="ones_k")
            nc.sync.dma_start(out=ones_k[:], in_=onesd[:].bitcast(f32r))
            ones_m = cpool.tile([1, NCLS], f32r, tag="ones_m")
            nc.sync.dma_start(
                out=ones_m[:], in_=onesd[:].rearrange("n o -> o n").bitcast(f32r)
            )

            def epilogue_a(state):
                """Softmax head for chunk c-1: exp via tanh + class-sum matmul."""
                psfc, c0 = state
                t_sb = epool.tile([NCLS, CHUNK], f32, tag="t")
                nc.scalar.activation(
                    t_sb[:], psfc[:], AF.Tanh, bias=fcbhs[:], scale=0.5
                )
                u = epool.tile([NCLS, CHUNK], f32, tag="u")
                nc.vector.tensor_scalar(u[:], t_sb[:], -1.0, 1.0, ALU.mult, ALU.add)
                r = epool.tile([NCLS, CHUNK], f32, tag="r")
                nc.vector.reciprocal(r[:], u[:])
                nc.vector.tensor_scalar_add(t_sb[:], t_sb[:], 1.0)
                e = epool.tile([NCLS, CHUNK], f32r, tag="e")
                with nc.allow_low_precision(reason="f32r is fp32-width for PE"):
                    nc.vector.tensor_mul(e[:], t_sb[:], r[:])
                pss = smpool.tile([1, CHUNK], f32, tag="pss")
                nc.tensor.matmul(pss[:], ones_k[:], e[:])
                rs = epool.tile([1, CHUNK], f32r, tag="rs")
                with nc.allow_low_precision(reason="f32r is fp32-width for PE"):
                    nc.vector.reciprocal(rs[:], pss[:])
                return (e, rs, c0)

            def epilogue_b(state):
                """Softmax tail for chunk c-2: broadcast 1/sum, scale, store."""
                e, rs, c0 = state
                psb = smpool.tile([NCLS, CHUNK], f32, tag="psb")
                nc.tensor.matmul(psb[:], ones_m[:], rs[:])
                prob = epool.tile([NCLS, CHUNK], f32, tag="prob")
                nc.vector.tensor_mul(prob[:], e[:].bitcast(f32), psb[:])
                nc.sync.dma_start(out=out_t[:, c0 : c0 + CHUNK], in_=prob[:])

            pend_a = None
            pend_b = None
            for c in range(NCH):
                c0 = c * CHUNK
                ac = prefetched.pop(c) if c in prefetched else load_a(c)

                # ---- layer 1 on DVE/ACT: gelu(w0*A0 + w1*A1 + b) ----
                h1 = h1pool.tile([P, KT, CHUNK], f32r, tag="h1")
                for m in range(KT):
                    tmp = epool.tile([P, CHUNK], f32, tag="l1t")
                    nc.vector.tensor_scalar_mul(
                        tmp[:], ac[:, 1, m, :], w1bs[:, m : m + 1]
                    )
                    pre = epool.tile([P, CHUNK], f32, tag="l1t")
                    nc.vector.scalar_tensor_tensor(
                        pre[:],
                        ac[:, 0, m, :],
                        w1as[:, m : m + 1],
                        tmp[:],
                        ALU.mult,
                        ALU.add,
                    )
                    nc.scalar.activation(
                        h1[:, m, :], pre[:], AF.Gelu, bias=bs[0][:, m : m + 1]
                    )

                # ---- layers 2/3 on PE ----
                src = h1
                for L in (2, 3):
                    h = hpool.tile([P, KT, CHUNK], f32r, tag=f"h{L}")
                    for m in range(KT):
                        ps = pspool.tile([P, CHUNK], f32, tag="ps")
                        for kt in range(KT):
                            nc.tensor.matmul(
                                ps[:],
                                ws[L][:, m, kt, :],
                                src[:, kt, :],
                                start=(kt == 0),
                                stop=(kt == KT - 1),
                            )
                        nc.scalar.activation(
                            h[:, m, :], ps[:], AF.Gelu, bias=bs[L - 1][:, m : m + 1]
                        )
                    src = h

                # dense head: logits^T = fc_w @ h3^T   -> [10, CHUNK] psum
                psfc = fcpool.tile([NCLS, CHUNK], f32, tag="psfc")
                for kt in range(KT):
                    nc.tensor.matmul(
                        psfc[:],
                        fcws[:, kt, :],
                        src[:, kt, :],
                        start=(kt == 0),
                        stop=(kt == KT - 1),
                    )

                new_b = epilogue_a(pend_a) if pend_a is not None else None
                if pend_b is not None:
                    epilogue_b(pend_b)
                pend_b = new_b
                pend_a = (psfc, c0)
            new_b = epilogue_a(pend_a)
            if pend_b is not None:
                epilogue_b(pend_b)
            epilogue_b(new_b)

    nc.compile()
    return nc


def _get_program():
    if "nc" not in _CACHE:
        _CACHE["nc"] = _build_program()
    return _CACHE["nc"]


def _dense_weight(idx: np.ndarray, w: np.ndarray) -> np.ndarray:
    """Scatter the [Dout, k] index/weight tables into a dense [Din, Dout]."""
    wd = np.zeros((D, D), dtype=np.float32)
    cols = np.arange(D)
    for j in range(idx.shape[1]):
        np.add.at(wd, (idx[:, j], cols), w[:, j])
    return wd


def kernel(**inputs) -> np.ndarray:
    x = np.asarray(inputs["x"], dtype=np.float32)
    idx1 = np.asarray(inputs["idx1"])
    idx2 = np.asarray(inputs["idx2"])
    idx3 = np.asarray(inputs["idx3"])
    w1 = np.asarray(inputs["w1"], dtype=np.float32)
    w2 = np.asarray(inputs["w2"], dtype=np.float32)
    w3 = np.asarray(inputs["w3"], dtype=np.float32)
    b1 = np.asarray(inputs["b1"], dtype=np.float32)
    b2 = np.asarray(inputs["b2"], dtype=np.float32)
    b3 = np.asarray(inputs["b3"], dtype=np.float32)
    fc_w = np.asarray(inputs["fc_w"], dtype=np.float32)
    fc_b = np.asarray(inputs["fc_b"], dtype=np.float32)

    nc = _get_program()

    def pack_w(wdense):
        # W[kt*P+p, m*P+n] -> [p, m, kt, n]
        return np.ascontiguousarray(
            wdense.reshape(KT, P, KT, P).transpose(1, 2, 0, 3)
        )

    def pack_vec(v):
        # v[m*P+p] -> [p, m]
        return v.reshape(KT, P).T

    tab = np.ascontiguousarray(
        np.stack(
            [pack_vec(b1), pack_vec(b2), pack_vec(b3), pack_vec(w1[:, 0]), pack_vec(w1[:, 1])],
            axis=2,
        )
    )
    shared = {
        "w2": pack_w(_dense_weight(idx2, w2)),
        "w3": pack_w(_dense_weight(idx3, w3)),
        "tab": tab,
        "fcwp": np.ascontiguousarray(
            fc_w.T.reshape(KT, P, NCLS).transpose(1, 0, 2)
        ),
        "fcbh": np.ascontiguousarray((fc_b / 2.0).reshape(NCLS, 1)),
        "onesd": np.ones((NCLS, 1), dtype=np.float32),
    }
    in_maps = []
    for i in range(NCORES):
        m = dict(shared)
        xsT = x[i * BS : (i + 1) * BS].T
        # a_j[kt*P+p, c*CHUNK+f] -> [p, c, j, kt, f]
        a = np.stack(
            [
                xsT[idx1[:, 0], :].reshape(KT, P, NCH, CHUNK).transpose(1, 2, 0, 3),
                xsT[idx1[:, 1], :].reshape(KT, P, NCH, CHUNK).transpose(1, 2, 0, 3),
            ],
            axis=2,
        )
        m["a01"] = np.ascontiguousarray(a)
        in_maps.append(m)

    from concourse.bass_utils import run_bass_kernel_spmd

    res = run_bass_kernel_spmd(nc, in_maps, list(range(NCORES)))
    kernel._last = res

    out = np.empty((B, NCLS), dtype=np.float32)
    for i, r in enumerate(res.results):
        out[i * BS : (i + 1) * BS] = r["out_t"].T
    return out


kernel._last = None
